# revision 1
# baseline (speedup 1.0000x reference)
"""CrystalGNN (GCNConv + mean-pool + FC + log_softmax) on 8 TRN2 NeuronCores.

Strategy (dst-range partitioned, dense normalized adjacency, v3):
- Core c owns dst nodes [c*1250, (c+1)*1250). Host builds the normalized
  adjacency block A_c[src, dst_local] = 16 * sum over edges (incl.
  self-loops) of dinv[src]*dinv[dst], shipped fp8-e4m3 (~12.3MB/core),
  pre-blocked bank-major (per PSUM bank, 78 full src blocks, partition =
  src % 128; the 16 leftover src rows ride in a tiny side tensor). The
  x16 scale keeps norms in e4m3's sweet spot; undone via bias*16 + relu
  + pool/16.
- Reformulation: out = (A^T x) W instead of A^T (x W). Phase 1 streams A
  as DoubleRow fp8 matmuls against x-blocks (fp8) as stationary,
  accumulating aggT[F, dst] per PSUM bank, plus one 16-row tail matmul
  per bank. Phase 2 per 128-dst chunk: PSUM preloaded with 16*b rows,
  aggT-chunk (bf16) stationary vs W moving accumulates on top, fused
  relu drain, then the pooling matmul.
- The whole A is SBUF-resident (~97KB/partition): all input DMAs are
  issued upfront (x split 3 ways first, then 4 A tiles per bank with a
  small leading tile) round-robin over the 3 DGE queues with no buffer
  recycling, so DMA engines free-run while PE chases completion
  semaphores. A few warmup matmuls on scratch data ramp the PE clock
  out of its low p-state during the DMA head.
- No collective: each core returns its partial pooled^T [128, 64]; the
  host sums the 8 partials and finishes FC + log_softmax (gather/unshard
  on host).
"""
import numpy as np
import ml_dtypes

N = 10000
E = 640000
F = 128
HD = 128
G = 64
NC = 8
PER = N // NC           # 1250
NBLK = 78               # full 128-row src blocks (39 DoubleRow pairs)
NFULL = NBLK * 128      # 9984
NTAIL = N - NFULL       # 16
BANKW = [512, 512, 226]
NBANK = 3
# src blocks per A tile within each bank (bank2 tapers so the stream's
# final tiles are tiny and all rings finish together)
TBLKSS = [[8, 8, 8, 8, 8, 8, 8, 8, 8, 6],
          [8, 8, 8, 8, 8, 8, 8, 8, 8, 6],
          [10, 10, 10, 10, 10, 10, 6, 6, 4, 2]]
# per-bank tile->DGE queue patterns, rotated so every ring gets equal bytes
TQS = [[0, 1, 2, 0, 1, 2, 0, 1, 2, 0],
       [1, 2, 0, 1, 2, 0, 1, 2, 0, 1],
       [2, 0, 1, 2, 0, 1, 2, 0, 1, 2]]
ASCALE = 16.0
NCHUNK = 10             # 128-dst chunks per core (last one 98 wide)
NWARM = 12

BF16 = ml_dtypes.bfloat16
F8 = ml_dtypes.float8_e4m3


def _plan(edge_index, batch_idx):
    src = edge_index[0].astype(np.int64)
    dst = edge_index[1].astype(np.int64)
    loops = np.arange(N, dtype=np.int64)
    src_f = np.concatenate([src, loops])
    dst_f = np.concatenate([dst, loops])

    deg = np.bincount(dst_f, minlength=N).astype(np.float64)
    dinv = 1.0 / np.sqrt(deg)
    wts = dinv[src_f] * dinv[dst_f] * ASCALE

    core_of = dst_f // PER
    A_ship = np.zeros((NC, 128, NBLK * PER), dtype=F8)
    A_tail = np.zeros((NC, NTAIL, PER), dtype=F8)
    for c in range(NC):
        m = core_of == c
        flat = src_f[m] * PER + (dst_f[m] - c * PER)
        A = np.bincount(flat, weights=wts[m], minlength=N * PER).reshape(N, PER)
        A3 = A[:NFULL].reshape(NBLK, 128, PER).transpose(1, 0, 2)  # [128, NBLK, PER]
        off = 0
        for o0, w in zip((0, 512, 1024), BANKW):
            A_ship[c, :, off:off + NBLK * w] = \
                A3[:, :, o0:o0 + w].reshape(128, NBLK * w).astype(F8)
            off += NBLK * w
        A_tail[c] = A[NFULL:].astype(F8)

    cnt = np.bincount(batch_idx.astype(np.int64), minlength=G).astype(np.float64)
    cnt = np.maximum(cnt, 1.0)
    mp = np.zeros((NC, 1280, G), dtype=np.float64)
    for c in range(NC):
        nodes = np.arange(c * PER, (c + 1) * PER)
        g = batch_idx[nodes].astype(np.int64)
        mp[c, np.arange(PER), g] = 1.0 / (cnt[g] * ASCALE)
    mp = mp.reshape(NC, 10, 128, G)
    mp = np.transpose(mp, (0, 2, 1, 3)).reshape(NC, 128, 10 * G).astype(np.float32)

    return dict(A_ship=A_ship, A_tail=A_tail, mpool=mp)


def _build():
    import concourse.bacc as bacc
    import concourse.mybir as mybir
    import concourse.tile as tile

    f32 = mybir.dt.float32
    bf16 = mybir.dt.bfloat16
    fp8 = mybir.dt.float8e4
    AF = mybir.ActivationFunctionType
    DR = mybir.MatmulPerfMode.DoubleRow

    nc = bacc.Bacc("TRN2", target_bir_lowering=False, debug=False, num_devices=NC)

    x_in = nc.dram_tensor("x_in", [128, NBLK * F], fp8, kind="ExternalInput")
    Amat = nc.dram_tensor("Amat", [128, NBLK * PER], fp8, kind="ExternalInput")
    # tail: x rows 9984..10000 ([:, :F]) then A tail rows bank-major ([:, F:])
    xa_tail = nc.dram_tensor("xa_tail", [NTAIL, F + PER], fp8, kind="ExternalInput")
    Wt = nc.dram_tensor("Wt", [F, HD], bf16, kind="ExternalInput")
    bb = nc.dram_tensor("bb", [128, HD], f32, kind="ExternalInput")  # rows of 16*b
    mpb = nc.dram_tensor("mpb", [128, NCHUNK * G], bf16, kind="ExternalInput")
    out = nc.dram_tensor("out", [128, 2 * G], f32, kind="ExternalOutput")

    with tile.TileContext(nc) as tc:
        with tc.tile_pool(name="const", bufs=1) as cp, \
             tc.tile_pool(name="aggp", bufs=1, space="PSUM") as aggp, \
             tc.tile_pool(name="pps", bufs=3, space="PSUM") as pps, \
             tc.tile_pool(name="poolp", bufs=1, space="PSUM") as poolp:

            qs = [nc.sync, nc.scalar, nc.gpsimd]

            # ---- upfront DMAs: each ring's issue order strictly matches PE
            # consumption order (x early, A tiles in bank/pair order, consts
            # in slack positions) so no tile arrives behind a later one ----
            x_sb = cp.tile([128, NBLK * F], fp8)
            xa_sb = cp.tile([NTAIL, F + PER], fp8)
            W_sb = cp.tile([F, HD], bf16)
            bb_sb = cp.tile([128, HD], f32)
            mp_sb = cp.tile([128, NCHUNK * G], bf16)
            NT = len(TBLKSS[0])
            a_tiles = [None] * (NBANK * NT)
            boffss = []
            for tb in TBLKSS:
                boffs = []
                acc = 0
                for nb in tb:
                    boffs.append(acc)
                    acc += nb
                boffss.append(boffs)

            def a_dma(bk, tix):
                w = BANKW[bk]
                nb = TBLKSS[bk][tix]
                boff = boffss[bk][tix]
                at = cp.tile([128, nb * w], fp8, name=f"at{bk}_{tix}")
                aoff = sum(NBLK * BANKW[b] for b in range(bk))
                qs[TQS[bk][tix]].dma_start(
                    at[:],
                    Amat[:, aoff + boff * w:aoff + (boff + nb) * w])
                a_tiles[bk * NT + tix] = at

            XCH = (NBLK // 6) * F          # 13 blocks per x chunk
            def x_dma(ci, r):
                lo = ci * XCH
                hi = (ci + 1) * XCH if ci < 5 else NBLK * F
                qs[r].dma_start(x_sb[:, lo:hi], x_in[:, lo:hi])

            sched = [
                ("x", 0, 0), ("x", 1, 1), ("x", 2, 2),
                ("a", 0, 0), ("a", 0, 1), ("a", 0, 2),
                ("x", 3, 0), ("x", 4, 1), ("x", 5, 2),
                ("xa",), ("a", 0, 3), ("a", 0, 4), ("a", 0, 5),
                ("a", 0, 6), ("a", 0, 7), ("a", 0, 8), ("a", 0, 9),
                ("W",), ("a", 1, 0), ("a", 1, 1), ("a", 1, 2),
                ("bb",), ("mpb",), ("a", 1, 3), ("a", 1, 4), ("a", 1, 5),
                ("a", 1, 6), ("a", 1, 7), ("a", 1, 8), ("a", 1, 9),
                ("a", 2, 0), ("a", 2, 1), ("a", 2, 2),
                ("a", 2, 3), ("a", 2, 4), ("a", 2, 5),
                ("a", 2, 6), ("a", 2, 7), ("a", 2, 8), ("a", 2, 9),
            ]
            for item in sched:
                if item[0] == "x":
                    x_dma(item[1], item[2])
                elif item[0] == "a":
                    a_dma(item[1], item[2])
                elif item[0] == "xa":
                    nc.gpsimd.dma_start(xa_sb[:], xa_tail[:])
                elif item[0] == "W":
                    nc.gpsimd.dma_start(W_sb[:], Wt[:])
                elif item[0] == "bb":
                    nc.gpsimd.dma_start(bb_sb[:], bb[:])
                elif item[0] == "mpb":
                    nc.gpsimd.dma_start(mp_sb[:], mpb[:])

            # ---- persistent SBUF staging ----
            aggT_sb = cp.tile([128, PER], bf16)       # x-aggregated, pre-W
            relu_sb = cp.tile([128, NCHUNK * HD], bf16)
            # two pooled accumulators (even/odd chunks) so the per-chunk
            # phase2->pool chains run in parallel; host sums the halves
            pooleds = [poolp.tile([128, G], f32, name="pooledp0"),
                       poolp.tile([128, G], f32, name="pooledp1")]

            NPAIR = NBLK // 2
            # pair j lives in tile pj_tile[bk][j] at local offset pj_off[bk][j]
            pj_tile = [[] for _ in range(NBANK)]
            pj_off = [[] for _ in range(NBANK)]
            for bk in range(NBANK):
                for tix, nb in enumerate(TBLKSS[bk]):
                    for lp in range(nb // 2):
                        pj_tile[bk].append(tix)
                        pj_off[bk].append(lp)

            chunk = 0
            for bk in range(NBANK):
                w = BANKW[bk]
                agg = aggp.tile([128, 512], f32, name=f"agg{bk}")
                for j in range(NPAIR):
                    at = a_tiles[bk * NT + pj_tile[bk][j]]
                    co = pj_off[bk][j] * 2 * w
                    nc.tensor.matmul(
                        agg[:, :w],
                        x_sb[:, (2 * j) * F:(2 * j + 2) * F]
                            .rearrange("p (k m) -> p k m", k=2),
                        at[:, co:co + 2 * w]
                            .rearrange("p (k n) -> p k n", k=2),
                        start=(j == 0),
                        stop=False,
                        skip_group_check=True,
                        perf_mode=DR,
                    )
                # 16 leftover src rows
                nc.tensor.matmul(
                    agg[:, :w],
                    xa_sb[:, :F],
                    xa_sb[:, F + bk * 512:F + bk * 512 + w],
                    start=False, stop=True,
                    skip_group_check=True,
                )
                # drain + post-process per 128-dst chunk (alternating engines
                # and accumulators so the chunk chains pipeline)
                base = bk * 512
                while chunk * 128 < base + w:
                    lo = chunk * 128
                    wt = min(128, PER - lo)
                    if chunk % 2 == 0:
                        nc.scalar.copy(aggT_sb[:, lo:lo + wt],
                                       agg[:, lo - base:lo - base + wt])
                    else:
                        nc.vector.tensor_copy(aggT_sb[:, lo:lo + wt],
                                              agg[:, lo - base:lo - base + wt])
                    po = pps.tile([128, HD], f32, tag="po")
                    nc.vector.tensor_copy(po[:wt, :], bb_sb[:wt, :])
                    nc.tensor.matmul(
                        po[:wt, :],
                        aggT_sb[:, lo:lo + wt],
                        W_sb[:],
                        start=False, stop=True,
                        skip_group_check=True,
                    )
                    nc.scalar.activation(
                        relu_sb[:wt, chunk * HD:(chunk + 1) * HD],
                        po[:wt, :], AF.Relu)
                    nc.tensor.matmul(
                        pooleds[chunk % 2][:],
                        relu_sb[:wt, chunk * HD:(chunk + 1) * HD],
                        mp_sb[:wt, chunk * G:(chunk + 1) * G],
                        start=(chunk < 2), stop=(chunk >= NCHUNK - 2),
                        skip_group_check=True,
                    )
                    chunk += 1

            pooled_sb = cp.tile([128, 2 * G], f32)
            nc.vector.tensor_copy(pooled_sb[:, :G], pooleds[0][:])
            nc.vector.tensor_copy(pooled_sb[:, G:], pooleds[1][:])
            nc.scalar.dma_start(out[:], pooled_sb[:])

    nc.compile()
    return nc


def _make_inputs(x, W, b, p):
    x = np.asarray(x, dtype=np.float32)
    xm = x[:NFULL].reshape(NBLK, 128, F).transpose(1, 0, 2).reshape(128, NBLK * F)
    bb = np.tile((np.asarray(b, dtype=np.float32) * ASCALE)[None, :], (128, 1))
    shared = dict(
        x_in=xm.astype(F8),
        Wt=np.asarray(W, dtype=np.float32).astype(BF16),
    )
    in_maps = []
    for c in range(NC):
        m = dict(shared)
        xa = np.zeros((NTAIL, F + PER), dtype=F8)
        xa[:, :F] = x[NFULL:].astype(F8)
        off = F
        a_t = p["A_tail"][c]
        for o0, w in zip((0, 512, 1024), BANKW):
            xa[:, off:off + w] = a_t[:, o0:o0 + w]
            off += w
        m["xa_tail"] = xa
        m["Amat"] = p["A_ship"][c]
        m["bb"] = bb
        m["mpb"] = p["mpool"][c].astype(BF16)
        in_maps.append(m)
    return in_maps


def _finish(results, W_fc, b_fc):
    pooledT = np.zeros((128, G), dtype=np.float64)
    for r in results:
        o = np.asarray(r["out"], dtype=np.float64)
        pooledT += o[:, :G] + o[:, G:]
    pooled = pooledT.T                                  # [G, HD]
    logits = pooled @ np.asarray(W_fc, np.float64) + np.asarray(b_fc, np.float64)
    t = logits - logits.max(axis=-1, keepdims=True)
    res = t - np.log(np.exp(t).sum(axis=-1, keepdims=True))
    return res.astype(np.float32)


def _run(x, edge_index, batch_idx, W, b, W_fc, b_fc, trace=False):
    from concourse.bass_utils import run_bass_kernel_spmd

    p = _plan(np.asarray(edge_index), np.asarray(batch_idx))
    nc = _build()
    in_maps = _make_inputs(x, W, b, p)
    res = run_bass_kernel_spmd(nc, in_maps, core_ids=list(range(NC)), trace=trace)
    return _finish(res.results, W_fc, b_fc), res


def kernel(x, edge_index, batch_idx, W, b, W_fc, b_fc):
    out, _ = _run(x, edge_index, batch_idx, W, b, W_fc, b_fc)
    return out

